# revision 16
# baseline (speedup 1.0000x reference)
"""Trainium2 Bass kernel for BinaryDecoderV2 — v4 (host-precomputed f8 weights).

Key insight over v3: the device-side weight unpack produced f8e4m3 weights
anyway, so the host can ship f8(-int_w) directly at the SAME byte count
(1 B/weight) — no vector nibble ops, no pack matmuls, no scalar casts.
Likewise int_sum is a pure function of the input true_sum, so the host
precomputes it and ships bf16 [batch, out] (8x smaller than f8 bit-planes).

Sharding: 4-way over batch x 2-way over out_features (per-core HBM:
latent 4.2MB + weights 4.2MB + int_sum 0.5MB ~= 8.9MB, vs 12.7MB in v3).

Device per core: stream k-ordered (weight, latent) tile groups; 128
DoubleRow fp8 matmuls accumulate -pred into 4 PSUM banks [128 out, 512
batch]; int_sum is injected mid-stream via 4 identity-lhsT bf16 matmuls
(PSUM then holds int_sum - pred = -255*diff); 4 vector tensor_tensor_reduce
ops square+sum each bank; host sums the 8x[128,4] partials.

Core c: out-shard c%2 (512 outputs), batch-shard c//2 (512 rows).
"""

import numpy as np
import ml_dtypes

IN_FEATURES = 8192
OUT_FEATURES = 1024
N_BITS = 8
BATCH = 2048
N_CORES = 8
OSH = 2                        # out-feature shards
BSH = 4                        # batch shards
OPC = OUT_FEATURES // OSH      # 512 outputs per core
BC = BATCH // BSH              # 512 batch rows per core
KP = 128
KT = IN_FEATURES // KP         # 64 k-subtiles
DKT = KT // 2                  # 32 DoubleRow k-tile rounds
NOH = OPC // 128               # 4 psum tiles (128 outputs each)
# k-group sizes (in kt units) for DMA chunking: small at the edges so the
# first matmul starts early and the last rounds wait on small transfers.
KGROUPS = [2, 2, 4, 8, 8, 8, 8, 8, 8, 4, 2, 2]
assert sum(KGROUPS) == KT
INJECT_ROUND = 8               # int_sum inject after this many DR rounds
SCALE = 2.0 ** N_BITS - 1.0
POWERS = [1.0, 2.0, 4.0, 8.0, 16.0, 32.0, 64.0, -128.0]

_CACHE: dict = {}


def _build():
    import concourse.bacc as bacc
    import concourse.mybir as mybir
    from concourse import tile

    f8e4 = mybir.dt.float8e4
    bf16 = mybir.dt.bfloat16
    f32 = mybir.dt.float32
    Act = mybir.ActivationFunctionType
    Alu = mybir.AluOpType
    PM = mybir.MatmulPerfMode

    nc = bacc.Bacc("TRN2", target_bir_lowering=False, debug=False,
                   num_devices=N_CORES)

    latq = nc.dram_tensor("latq", [128, KT, BC], f8e4, kind="ExternalInput")
    w8f = nc.dram_tensor("w8f", [128, KT, OPC], f8e4, kind="ExternalInput")
    ints = nc.dram_tensor("ints", [128, NOH, BC], bf16, kind="ExternalInput")
    dg = nc.dram_tensor("dg", [128, 128], bf16, kind="ExternalInput")
    partials = nc.dram_tensor("partials", [128, NOH], f32,
                              kind="ExternalOutput")

    with tile.TileContext(nc) as tc:
        with (
            tc.tile_pool(name="wp", bufs=1) as w_pool,
            tc.tile_pool(name="lp", bufs=1) as l_pool,
            tc.tile_pool(name="cst", bufs=1) as cst_pool,
            tc.tile_pool(name="out", bufs=1) as out_pool,
            tc.tile_pool(name="ps", bufs=1, space="PSUM") as psum_pool,
        ):
            # ---- all input DMAs issued up-front, spread over four engines
            # (vector/sync free up earliest after the NEFF preamble);
            # dedicated tiles (no pool-reuse semaphores); k-interleaved
            # group order so round r's weight+latent tiles land together.
            # ints (0.5MB, not needed until INJECT_ROUND) issues from
            # scalar so it can't delay the k-stream. ----
            dgt = cst_pool.tile([128, 128], bf16, name="dgt", tag="dgt")
            intt = cst_pool.tile([128, NOH, BC], bf16, name="intt", tag="intt")
            wts, lts = [], []
            kt0 = 0
            for gi, n in enumerate(KGROUPS):
                wt = w_pool.tile([128, n, OPC], f8e4, name=f"w{gi}",
                                 tag=f"w{gi}")
                lt = l_pool.tile([128, n, BC], f8e4, name=f"l{gi}",
                                 tag=f"l{gi}")
                nc.sync.dma_start(wt[:], w8f[:, kt0:kt0 + n, :])
                nc.gpsimd.dma_start(lt[:], latq[:, kt0:kt0 + n, :])
                wts.append((wt, kt0))
                lts.append((lt, kt0))
                kt0 += n
            nc.sync.dma_start(dgt[:], dg[:])
            nc.gpsimd.dma_start(intt[:], ints[:])

            psums = [psum_pool.tile([128, BC], f32, name=f"ps{i}",
                                    tag=f"ps{i}") for i in range(NOH)]
            out_t = out_pool.tile([128, NOH], f32, name="out_t", tag="out_t")
            sqs = [out_pool.tile([128, BC], f32, name=f"sq{i}", tag=f"sq{i}")
                   for i in range(NOH)]
            sq2s = [out_pool.tile([128, BC], f32, name=f"sq2_{i}",
                                  tag=f"sq2_{i}") for i in range(2)]

            # ---- main matmul stream: psum[oh] = int_sum - pred ----
            gi = 0
            for r in range(DKT):
                kt = 2 * r
                wt, wbase = wts[gi]
                lt, lbase = lts[gi]
                if kt - wbase >= KGROUPS[gi] :
                    gi += 1
                    wt, wbase = wts[gi]
                    lt, lbase = lts[gi]
                a = kt - wbase
                last = (r == DKT - 1)
                for oh in range(NOH):
                    nc.tensor.matmul(
                        psums[oh][:],
                        wt[:, a:a + 2, oh * 128:(oh + 1) * 128],
                        lt[:, a:a + 2, :],
                        start=(r == 0), stop=last,
                        perf_mode=PM.DoubleRow)
                    if last:
                        # square+reduce tail split over two engines:
                        # scalar (Square w/ accumulator) for oh 0-1,
                        # vector (copy out of PSUM, then ttr) for oh 2-3
                        nc.scalar.activation(
                            sqs[oh][:], psums[oh][:], Act.Square,
                            accum_out=out_t[:, oh:oh + 1])
                if r == INJECT_ROUND:
                    for oh in range(NOH):
                        nc.tensor.matmul(
                            psums[oh][:], dgt[:],
                            intt[:, oh, :],
                            start=False, stop=False)

            nc.sync.dma_start(partials[:], out_t[:])

    nc.compile()
    return nc


def _get_nc():
    if "nc" not in _CACHE:
        _CACHE["nc"] = _build()
    return _CACHE["nc"]


def make_in_maps(latent: np.ndarray, true_sum: np.ndarray,
                 weight: np.ndarray) -> list:
    f8 = ml_dtypes.float8_e4m3fn
    bf = ml_dtypes.bfloat16

    # latq per batch shard: latq[p, kt, n] = latent[sb*BC + n, kt*128 + p]
    lat8 = latent.astype(f8)
    latqs = []
    for sb in range(BSH):
        ls = lat8[sb * BC:(sb + 1) * BC, :]
        latqs.append(np.ascontiguousarray(
            ls.T.reshape(KT, KP, BC).transpose(1, 0, 2)))

    # int weights from sign bits; ship f8(-int_w) directly
    bits = (weight > 0).reshape(IN_FEATURES, OUT_FEATURES, N_BITS)
    pw = np.asarray(POWERS, dtype=np.float32)
    int_w = bits.astype(np.float32) @ pw          # [in, out]
    w8_full = (-int_w).astype(f8)
    w8fs = []
    for so in range(OSH):
        wcol = w8_full[:, so * OPC:(so + 1) * OPC]
        w8fs.append(np.ascontiguousarray(
            wcol.reshape(KT, KP, OPC).transpose(1, 0, 2)))

    # int_sum precomputed exactly, shipped bf16
    int_sum = (true_sum.reshape(BATCH, OUT_FEATURES, N_BITS)
               .astype(np.float32) @ pw)          # [batch, out]
    ints_bf = int_sum.astype(bf)

    dgm = np.eye(128, dtype=np.float32).astype(bf)

    in_maps = []
    for c in range(N_CORES):
        so, sb = c % OSH, c // OSH
        # ints[o128, oh, n] = int_sum[sb*BC+n, so*OPC + oh*128 + o128]
        S = ints_bf[sb * BC:(sb + 1) * BC, so * OPC:(so + 1) * OPC]
        ic = np.ascontiguousarray(
            S.reshape(BC, NOH, 128).transpose(2, 1, 0))
        in_maps.append({"latq": latqs[sb], "w8f": w8fs[so], "ints": ic,
                        "dg": dgm})
    return in_maps


def kernel(latent: np.ndarray, true_sum: np.ndarray,
           weight: np.ndarray) -> np.ndarray:
    from concourse.bass_utils import run_bass_kernel_spmd

    nc = _get_nc()
    in_maps = make_in_maps(latent, true_sum, weight)
    res = run_bass_kernel_spmd(nc, in_maps, list(range(N_CORES)))

    total = 0.0
    for c in range(N_CORES):
        total += float(res.results[c]["partials"].astype(np.float64).sum())
    loss = total / (BATCH * OUT_FEATURES) / (SCALE * SCALE)
    return np.array(loss, dtype=np.float32)


# revision 17
# speedup vs baseline: 1.2862x; 1.2862x over previous
"""Trainium2 Bass kernel for BinaryDecoderV2 — v4 (host-precomputed f8 weights).

Key insight over v3: the device-side weight unpack produced f8e4m3 weights
anyway, so the host can ship f8(-int_w) directly at the SAME byte count
(1 B/weight) — no vector nibble ops, no pack matmuls, no scalar casts.
Likewise int_sum is a pure function of the input true_sum, so the host
precomputes it and ships bf16 [batch, out] (8x smaller than f8 bit-planes).

Sharding: 4-way over batch x 2-way over out_features (per-core HBM:
latent 4.2MB + weights 4.2MB + int_sum 0.5MB ~= 8.9MB, vs 12.7MB in v3).

Device per core: stream k-ordered (weight, latent) tile groups; 128
DoubleRow fp8 matmuls accumulate -pred into 4 PSUM banks [128 out, 512
batch]; int_sum is injected mid-stream via 4 identity-lhsT bf16 matmuls
(PSUM then holds int_sum - pred = -255*diff); 4 vector tensor_tensor_reduce
ops square+sum each bank; host sums the 8x[128,4] partials.

Core c: out-shard c%2 (512 outputs), batch-shard c//2 (512 rows).
"""

import numpy as np
import ml_dtypes

IN_FEATURES = 8192
OUT_FEATURES = 1024
N_BITS = 8
BATCH = 2048
N_CORES = 8
OSH = 2                        # out-feature shards
BSH = 4                        # batch shards
OPC = OUT_FEATURES // OSH      # 512 outputs per core
BC = BATCH // BSH              # 512 batch rows per core
KP = 128
KT = IN_FEATURES // KP         # 64 k-subtiles
DKT = KT // 2                  # 32 DoubleRow k-tile rounds
NOH = OPC // 128               # 4 psum tiles (128 outputs each)
# k-group sizes (in kt units) for DMA chunking: small at the edges so the
# first matmul starts early and the last rounds wait on small transfers.
KGROUPS = [2, 2, 4, 8, 8, 8, 8, 8, 8, 4, 2, 2]
assert sum(KGROUPS) == KT
INJECT_ROUND = 8               # int_sum inject after this many DR rounds
SCALE = 2.0 ** N_BITS - 1.0
POWERS = [1.0, 2.0, 4.0, 8.0, 16.0, 32.0, 64.0, -128.0]

_CACHE: dict = {}


def _build():
    import concourse.bacc as bacc
    import concourse.mybir as mybir
    from concourse import tile

    f8e4 = mybir.dt.float8e4
    bf16 = mybir.dt.bfloat16
    f32 = mybir.dt.float32
    Act = mybir.ActivationFunctionType
    Alu = mybir.AluOpType
    PM = mybir.MatmulPerfMode

    nc = bacc.Bacc("TRN2", target_bir_lowering=False, debug=False,
                   num_devices=N_CORES)

    latq = nc.dram_tensor("latq", [128, KT, BC], f8e4, kind="ExternalInput")
    w8f = nc.dram_tensor("w8f", [128, KT, OPC], f8e4, kind="ExternalInput")
    ints = nc.dram_tensor("ints", [128, NOH, BC], bf16, kind="ExternalInput")
    dg = nc.dram_tensor("dg", [128, 128], bf16, kind="ExternalInput")
    partials = nc.dram_tensor("partials", [128, NOH], f32,
                              kind="ExternalOutput")

    with tile.TileContext(nc) as tc:
        with (
            tc.tile_pool(name="wp", bufs=1) as w_pool,
            tc.tile_pool(name="lp", bufs=1) as l_pool,
            tc.tile_pool(name="cst", bufs=1) as cst_pool,
            tc.tile_pool(name="out", bufs=1) as out_pool,
            tc.tile_pool(name="ps", bufs=1, space="PSUM") as psum_pool,
        ):
            # ---- all input DMAs issued up-front, spread over four engines
            # (vector/sync free up earliest after the NEFF preamble);
            # dedicated tiles (no pool-reuse semaphores); k-interleaved
            # group order so round r's weight+latent tiles land together.
            # ints (0.5MB, not needed until INJECT_ROUND) issues from
            # scalar so it can't delay the k-stream. ----
            dgt = cst_pool.tile([128, 128], bf16, name="dgt", tag="dgt")
            intt = cst_pool.tile([128, NOH, BC], bf16, name="intt", tag="intt")
            nc.scalar.dma_start(dgt[:], dg[:])
            nc.scalar.dma_start(intt[:], ints[:])
            wts, lts = [], []
            kt0 = 0
            for gi, n in enumerate(KGROUPS):
                wt = w_pool.tile([128, n, OPC], f8e4, name=f"w{gi}",
                                 tag=f"w{gi}")
                lt = l_pool.tile([128, n, BC], f8e4, name=f"l{gi}",
                                 tag=f"l{gi}")
                nc.sync.dma_start(wt[:], w8f[:, kt0:kt0 + n, :])
                nc.gpsimd.dma_start(lt[:], latq[:, kt0:kt0 + n, :])
                wts.append((wt, kt0))
                lts.append((lt, kt0))
                kt0 += n

            psums = [psum_pool.tile([128, BC], f32, name=f"ps{i}",
                                    tag=f"ps{i}") for i in range(NOH)]
            out_t = out_pool.tile([128, NOH], f32, name="out_t", tag="out_t")
            sqs = [out_pool.tile([128, BC], f32, name=f"sq{i}", tag=f"sq{i}")
                   for i in range(NOH)]
            sq2s = [out_pool.tile([128, BC], f32, name=f"sq2_{i}",
                                  tag=f"sq2_{i}") for i in range(2)]

            # ---- main matmul stream: psum[oh] = int_sum - pred ----
            gi = 0
            for r in range(DKT):
                kt = 2 * r
                wt, wbase = wts[gi]
                lt, lbase = lts[gi]
                if kt - wbase >= KGROUPS[gi] :
                    gi += 1
                    wt, wbase = wts[gi]
                    lt, lbase = lts[gi]
                a = kt - wbase
                last = (r == DKT - 1)
                for oh in range(NOH):
                    nc.tensor.matmul(
                        psums[oh][:],
                        wt[:, a:a + 2, oh * 128:(oh + 1) * 128],
                        lt[:, a:a + 2, :],
                        start=(r == 0), stop=last,
                        perf_mode=PM.DoubleRow)
                    if last:
                        # square+reduce tail split over two engines:
                        # scalar (Square w/ accumulator) for oh 0-1,
                        # vector (copy out of PSUM, then ttr) for oh 2-3
                        nc.scalar.activation(
                            sqs[oh][:], psums[oh][:], Act.Square,
                            accum_out=out_t[:, oh:oh + 1])
                if r == INJECT_ROUND:
                    for oh in range(NOH):
                        nc.tensor.matmul(
                            psums[oh][:], dgt[:],
                            intt[:, oh, :],
                            start=False, stop=False)

            nc.sync.dma_start(partials[:], out_t[:])

    nc.compile()
    return nc


def _get_nc():
    if "nc" not in _CACHE:
        _CACHE["nc"] = _build()
    return _CACHE["nc"]


def make_in_maps(latent: np.ndarray, true_sum: np.ndarray,
                 weight: np.ndarray) -> list:
    f8 = ml_dtypes.float8_e4m3fn
    bf = ml_dtypes.bfloat16

    # latq per batch shard: latq[p, kt, n] = latent[sb*BC + n, kt*128 + p]
    lat8 = latent.astype(f8)
    latqs = []
    for sb in range(BSH):
        ls = lat8[sb * BC:(sb + 1) * BC, :]
        latqs.append(np.ascontiguousarray(
            ls.T.reshape(KT, KP, BC).transpose(1, 0, 2)))

    # int weights from sign bits; ship f8(-int_w) directly
    bits = (weight > 0).reshape(IN_FEATURES, OUT_FEATURES, N_BITS)
    pw = np.asarray(POWERS, dtype=np.float32)
    int_w = bits.astype(np.float32) @ pw          # [in, out]
    w8_full = (-int_w).astype(f8)
    w8fs = []
    for so in range(OSH):
        wcol = w8_full[:, so * OPC:(so + 1) * OPC]
        w8fs.append(np.ascontiguousarray(
            wcol.reshape(KT, KP, OPC).transpose(1, 0, 2)))

    # int_sum precomputed exactly, shipped bf16
    int_sum = (true_sum.reshape(BATCH, OUT_FEATURES, N_BITS)
               .astype(np.float32) @ pw)          # [batch, out]
    ints_bf = int_sum.astype(bf)

    dgm = np.eye(128, dtype=np.float32).astype(bf)

    in_maps = []
    for c in range(N_CORES):
        so, sb = c % OSH, c // OSH
        # ints[o128, oh, n] = int_sum[sb*BC+n, so*OPC + oh*128 + o128]
        S = ints_bf[sb * BC:(sb + 1) * BC, so * OPC:(so + 1) * OPC]
        ic = np.ascontiguousarray(
            S.reshape(BC, NOH, 128).transpose(2, 1, 0))
        in_maps.append({"latq": latqs[sb], "w8f": w8fs[so], "ints": ic,
                        "dg": dgm})
    return in_maps


def kernel(latent: np.ndarray, true_sum: np.ndarray,
           weight: np.ndarray) -> np.ndarray:
    from concourse.bass_utils import run_bass_kernel_spmd

    nc = _get_nc()
    in_maps = make_in_maps(latent, true_sum, weight)
    res = run_bass_kernel_spmd(nc, in_maps, list(range(N_CORES)))

    total = 0.0
    for c in range(N_CORES):
        total += float(res.results[c]["partials"].astype(np.float64).sum())
    loss = total / (BATCH * OUT_FEATURES) / (SCALE * SCALE)
    return np.array(loss, dtype=np.float32)
